# revision 33
# baseline (speedup 1.0000x reference)
"""Block-sparse MoE (SwiGLU, top-k of 8 experts) on 8 Trainium2 NeuronCores.

Sharding: ffn-dimension-parallel (the spec's primary hint). Every core holds
a F/8 = 512-row slice of w1/w3/w2 for ALL 8 experts; x is replicated
(gathered per expert on the host, pre-scaled capacity C_e per expert).
Each core loops over experts e: phase1(e) computes its slice of
inter = silu(w1 x) * (w3 x); phase2(e) computes the partial output
(w2_slice.T @ inter) * wgt. The host sums the 8 partial outputs
(the "all-reduce") and scatter-adds into the full [T, H] result.

Why this beats expert-parallel (core e = expert e): EP pads every core to
max_e count_e (556 slots for seed-0 routing), ffn-parallel gives every core
identical work of sum_e C_e ~ 4104 slots at 1/8 the ffn width - a ~7%
PE-cycle reduction, robust to any routing skew.

Device loop is software-pipelined as [phase1(e); phase2(e-1)] so the
PSUM->SBUF drain of inter(e) hides under a full phase, never bubbling the
PE. Partial outputs are stored as fp16 (quantization ~5e-7 of out scale
per partial; measured end-to-end rel err stays ~5e-4) to halve the output
DMA. Matmuls default to fp16 as in the EP baseline.
All weight/x DRAM->SBUF transfers are host-pre-tiled contiguous DMAs.
"""

import math
import os

import numpy as np

H = 2048            # hidden dim
F = 4096            # ffn dim per expert
E = 8               # experts
NCORES = 8
P = 128             # partitions
NH = H // P         # 16 h-tiles
FSH = F // NCORES   # 512 ffn rows per core
NFS = FSH // P      # 4 f-tiles per expert per core

DT_MODE = os.environ.get("MOE_DT", "fp16")     # fp16 | bf16 | f32r | f32
OUT_DT = os.environ.get("MOE_OUT_DT", "fp16")  # fp16 | f32 partial outputs

# populated by kernel() for test harness introspection
LAST_STATS = {}

_BUILD_CACHE = {}


def _chunk_plan(count):
    """(nch, w): capacity C_e = nch*w >= count, even chunk width w <= 512
    (PSUM bank is 512 fp32; f32r matmuls need an even moving dim)."""
    if count <= 0:
        return (0, 0)
    c_min = max(64, count)
    n = max(1, math.ceil(c_min / 512))
    w = 2 * math.ceil(c_min / (2 * n))
    return (n, w)


def _build(plan, dt_mode, out_dt):
    """Build + compile the per-core Bass program.

    plan: tuple of E (nch, w) chunk shapes, one per expert (nch=0 -> skip).
    """
    import concourse.bacc as bacc
    import concourse.mybir as mybir
    from concourse import tile

    AF = mybir.ActivationFunctionType
    f32 = mybir.dt.float32
    dmm = {
        "bf16": mybir.dt.bfloat16,
        "fp16": mybir.dt.float16,
        "f32": f32,
    }.get(dt_mode, mybir.dt.float32r)
    odt = f32 if out_dt == "f32" else mybir.dt.float16

    order = [e for e in range(E) if plan[e][0] > 0]
    caps = {e: plan[e][0] * plan[e][1] for e in order}
    offs = {}
    ctot = 0
    for e in order:
        offs[e] = ctot
        ctot += caps[e]

    nc = bacc.Bacc("TRN2", target_bir_lowering=False, debug=False)

    # Host-pre-tiled DRAM layouts (every DMA below is fully contiguous):
    #   xt{e}  [nch, P, NH, w]      xt[ci, p, n, c] = x_tok[ci*w+c, n*P+p]
    #   w13t   [E, NFS, P, 2, NH, P] [e,fi,p,m,n,j] = w{1,3}slice[e, fi*P+j, n*P+p]
    #   w2t    [E, P, NH, NFS, P]    [e,p,ht,fi,j]  = w2slice[e, fi*P+p, ht*P+j]
    #   wg{e}  [P, C_e]             broadcast routing weights
    #   yt     [H, Ctot]            partial output, column block off_e per expert
    xt_d = {
        e: nc.dram_tensor(f"xt{e}", [plan[e][0], P, NH, plan[e][1]], dmm,
                          kind="ExternalInput").ap()
        for e in order
    }
    w13_d = nc.dram_tensor("w13t", [E, NFS, P, 2, NH, P], dmm,
                           kind="ExternalInput").ap()
    w2_d = nc.dram_tensor("w2t", [E, P, NH, NFS, P], dmm,
                          kind="ExternalInput").ap()
    wg_d = {
        e: nc.dram_tensor(f"wg{e}", [P, caps[e]], f32,
                          kind="ExternalInput").ap()
        for e in order
    }
    # per-expert outputs [NH/2, P, 2, C_e]: every store DMA is one fully
    # contiguous burst covering TWO h-tiles (row-strided stores into a
    # [H, Ctot] tensor poison the shared DMA engines and starve the
    # weight stream; per-ht stores pay too much SWDGE issue cost)
    y_d = {
        e: nc.dram_tensor(f"yt{e}", [NH // 2, P, 2, caps[e]], odt,
                          kind="ExternalOutput").ap()
        for e in order
    }

    with tile.TileContext(nc) as tc:
        with (
            tc.tile_pool(name="inter", bufs=2) as inter_pool,
            tc.tile_pool(name="psum", bufs=4, space="PSUM") as psum_pool,
            tc.tile_pool(name="misc", bufs=1) as misc_pool,
            tc.tile_pool(name="xtp", bufs=4) as xt_pool,
            tc.tile_pool(name="wcol", bufs=5) as wcol_pool,
            tc.tile_pool(name="p1tmp", bufs=2) as p1tmp,
            tc.tile_pool(name="w2col", bufs=2) as w2_pool,
            tc.tile_pool(name="wgp", bufs=3) as wg_pool,
            # ob ring must cover the SWDGE store latency (~6-12 us) at the
            # ~1.7 us/pair drain cadence, else the DVE drain stalls and
            # the PE blocks on PSUM slot reuse behind it.
            tc.tile_pool(name="obp", bufs=8) as ob_pool,
        ):
            # PE warmup: zero-matmuls with no DMA dependencies run
            # immediately, lifting the HAM clock gate (1.2 -> 2.4 GHz)
            # while the first real loads are still in flight.
            wsrc = misc_pool.tile([P, P], dmm, tag="wsrc")
            nc.vector.memset(wsrc[:], 0.0)
            wps = psum_pool.tile([P, 64], f32, tag="ps3", bufs=4,
                                 name="warm_ps")
            for i in range(80):
                nc.tensor.matmul(wps[:], wsrc[:], wsrc[:, :64],
                                 start=(i == 0), stop=(i == 79))

            # Startup critical path: the SP queue issues its first DMA
            # earliest (~6 us). Interleave the first expert's chunk-a and
            # the first weight column on SP in h-halves so the hi<8
            # matmuls start as soon as the first halves land. The second
            # expert's x rides the ACT HWDGE queue (starts ~9-10 us).
            e0 = order[0]
            xts = {}

            def _xt_tiles(e):
                nch, w = plan[e]
                return [
                    xt_pool.tile([P, NH, w], dmm, tag="xt",
                                 name=f"xt{e}_{ci}")
                    for ci in range(nch)
                ]

            xts[e0] = _xt_tiles(e0)
            wc0 = wcol_pool.tile([P, 2, NH, P], dmm, tag="wc", name="wc0")
            # early-window DMA rate is ~135 GB/s PER QUEUE (instruction
            # fetch streams), and the queues are independent: split the
            # startup-critical path across both HWDGE queues — weights on
            # SP, x on ACT — in graded h-pieces (2,2,4,8 h-tiles) so the
            # first matmul starts after ~0.5 MB and the hi-loop consumes
            # later pieces as they land.
            pieces = [(0, 2), (2, 2), (4, 4), (8, 8)]
            for p0, pw in pieces:
                sl = slice(p0, p0 + pw)
                nc.sync.dma_start(wc0[:, :, sl, :], w13_d[e0][0][:, :, sl, :])
                nc.scalar.dma_start(xts[e0][0][:, sl, :],
                                    xt_d[e0][0][:, sl, :])
            for ci in range(1, plan[e0][0]):
                for sl in (slice(0, 4), slice(4, 10), slice(10, NH)):
                    nc.scalar.dma_start(xts[e0][ci][:, sl, :],
                                        xt_d[e0][ci][:, sl, :])
            if len(order) > 1:
                e1 = order[1]
                xts[e1] = _xt_tiles(e1)
                for ci in range(plan[e1][0]):
                    nc.scalar.dma_start(xts[e1][ci][:], xt_d[e1][ci])

            prev = None  # (e, inter tiles, w2c, wg, chunks)
            wc_anchor = None

            for idx, e in enumerate(order):
                nch, w = plan[e]
                C = caps[e]
                chunks = [(ci * w, w) for ci in range(nch)]

                # prefetch the next expert's x (e0/e1 preloaded at startup)
                if idx + 1 < len(order):
                    en = order[idx + 1]
                    if en not in xts:
                        xts[en] = _xt_tiles(en)
                        for ci in range(plan[en][0]):
                            nc.scalar.dma_start(xts[en][ci][:], xt_d[en][ci])

                # routing weights for this expert (consumed one block later
                # in phase2(e)); SWDGE queue, issued after the fi loop so
                # it never competes with this expert's weight columns.
                wg = wg_pool.tile([P, C], f32, tag="wg", name=f"wg{e}_t")

                # ---- phase 1: inter[f', :] = silu(w1 @ xT) * (w3 @ xT) ----
                its = []
                for fi in range(NFS):
                    if idx == 0 and fi == 0:
                        wc = wc0        # preloaded in the startup block
                    else:
                        wc = wcol_pool.tile([P, 2, NH, P], dmm, tag="wc",
                                            name=f"wc{e}_{fi}")
                        if idx == 0 and fi <= 2:
                            # still inside the slow startup window: land
                            # in quarters so the hi-loop can start early
                            h4 = NH // 4
                            for q in range(4):
                                sl = slice(q * h4, (q + 1) * h4)
                                dma = nc.sync.dma_start(
                                    wc[:, :, sl, :],
                                    w13_d[e][fi][:, :, sl, :])
                                if fi == 1:
                                    wc_anchor = dma
                        else:
                            nc.sync.dma_start(wc[:], w13_d[e][fi])
                    it = inter_pool.tile([P, C], dmm, tag=f"inter{fi}",
                                         name=f"inter{e}_{fi}")
                    its.append(it)
                    if idx == 0 and fi == 0:
                        # chunk-outer: chunk-a's matmuls only wait on the
                        # first x chunk's DMA
                        for ci, (c0, cw) in enumerate(chunks):
                            ps1s = psum_pool.tile([P, cw], f32, tag="ps1",
                                                  bufs=4, name=f"ps1s{ci}")
                            ps3s = psum_pool.tile([P, cw], f32, tag="ps3",
                                                  bufs=4, name=f"ps3s{ci}")
                            for hi in range(NH):
                                nc.tensor.matmul(
                                    ps1s[:], wc[:, 0, hi, :],
                                    xts[e][ci][:, hi, :],
                                    start=(hi == 0), stop=(hi == NH - 1))
                            for hi in range(NH):
                                nc.tensor.matmul(
                                    ps3s[:], wc[:, 1, hi, :],
                                    xts[e][ci][:, hi, :],
                                    start=(hi == 0), stop=(hi == NH - 1))
                            sig = p1tmp.tile([P, cw], f32, tag="sig")
                            nc.scalar.activation(sig[:], ps1s[:], AF.Sigmoid)
                            sil = p1tmp.tile([P, cw], f32, tag="sil")
                            nc.vector.tensor_mul(sil[:], ps1s[:], sig[:])
                            nc.vector.tensor_mul(it[:, c0:c0 + cw], sil[:],
                                                 ps3s[:])
                    else:
                        # interleaved: consecutive matmuls share the
                        # stationary operand across chunks
                        ps1 = [psum_pool.tile([P, cw], f32, tag="ps1",
                                              bufs=4, name=f"ps1_{e}_{fi}_{ci}")
                               for ci, (c0, cw) in enumerate(chunks)]
                        ps3 = [psum_pool.tile([P, cw], f32, tag="ps3",
                                              bufs=4, name=f"ps3_{e}_{fi}_{ci}")
                               for ci, (c0, cw) in enumerate(chunks)]
                        for hi in range(NH):
                            for ci in range(nch):
                                nc.tensor.matmul(
                                    ps1[ci][:], wc[:, 0, hi, :],
                                    xts[e][ci][:, hi, :],
                                    start=(hi == 0), stop=(hi == NH - 1))
                            for ci in range(nch):
                                nc.tensor.matmul(
                                    ps3[ci][:], wc[:, 1, hi, :],
                                    xts[e][ci][:, hi, :],
                                    start=(hi == 0), stop=(hi == NH - 1))
                        for ci, (c0, cw) in enumerate(chunks):
                            sig = p1tmp.tile([P, cw], f32, tag="sig")
                            nc.scalar.activation(sig[:], ps1[ci][:],
                                                 AF.Sigmoid)
                            sil = p1tmp.tile([P, cw], f32, tag="sil")
                            nc.vector.tensor_mul(sil[:], ps1[ci][:], sig[:])
                            nc.vector.tensor_mul(it[:, c0:c0 + cw], sil[:],
                                                 ps3[ci][:])

                # w2 slice for this expert: one contiguous 2.1 MB DMA on
                # the SWDGE (gpsimd) queue. The early-window DMA budget is
                # GLOBAL across queues (~130 GB/s total): w2/wg for the
                # first expert aren't needed until its phase 2 (~65 us),
                # so gate them behind the startup-critical weight columns.
                wg_dma = nc.gpsimd.dma_start(wg[:], wg_d[e][:])
                w2c = w2_pool.tile([P, NH, NFS, P], dmm, tag="w2c",
                                   name=f"w2c{e}")
                w2_dma = nc.gpsimd.dma_start(w2c[:], w2_d[e])
                if idx == 0 and wc_anchor is not None:
                    tile.add_dep_helper(
                        wg_dma.ins, wc_anchor.ins,
                        reason="delay wg past kernel startup")
                    tile.add_dep_helper(
                        w2_dma.ins, wc_anchor.ins,
                        reason="delay w2 load past kernel startup")

                # ---- phase 2 of the previous expert ----
                if prev is not None:
                    _phase2(nc, tc, psum_pool, ob_pool, y_d, prev,
                            last=False, odt=odt)
                prev = (e, its, w2c, wg, chunks, offs[e], C)

            _phase2(nc, tc, psum_pool, ob_pool, y_d, prev, last=True,
                    odt=odt)

    nc.compile()
    return nc


def _phase2(nc, tc, psum_pool, ob_pool, y_d, prev, last, odt):
    """yT[ht, :] = (w2slice.T @ interT) * wgt for one expert."""
    import concourse.mybir as mybir
    f32 = mybir.dt.float32
    e, its, w2c, wg, chunks, off, C = prev
    nch = len(chunks)
    for hp in range(NH // 2):
        ob = ob_pool.tile([P, 2, C], odt, tag="ob", name=f"ob{e}_{hp}")
        for m in range(2):
            ht = 2 * hp + m
            # po shares the (phase-1) ps1 tag: 4 PSUM slots total keep
            # the next group's matmuls from waiting on the drain.
            po = [psum_pool.tile([P, cw], f32, tag="ps1", bufs=4,
                                 name=f"po_{e}_{ht}_{ci}")
                  for ci, (c0, cw) in enumerate(chunks)]
            for fi in range(NFS):
                for ci, (c0, cw) in enumerate(chunks):
                    nc.tensor.matmul(
                        po[ci][:], w2c[:, ht, fi, :],
                        its[fi][:, c0:c0 + cw],
                        start=(fi == 0), stop=(fi == NFS - 1))
            for ci, (c0, cw) in enumerate(chunks):
                nc.vector.tensor_mul(ob[:, m, c0:c0 + cw], po[ci][:],
                                     wg[:, c0:c0 + cw])
            if last and hp >= NH // 2 - 2:
                # final pairs: store per h-tile, alternating queues, so
                # the tail drain overlaps the last matmul groups
                eng = nc.gpsimd if m == 0 else nc.scalar
                eng.dma_start(y_d[e][hp][:, m], ob[:, m])
        if not (last and hp >= NH // 2 - 2):
            # alternate queues: halves each store queue's backlog, so the
            # end-of-kernel queue drains are short
            eng = nc.gpsimd if hp % 2 == 0 else nc.scalar
            eng.dma_start(y_d[e][hp], ob[:])


def _get_nc(plan, dt_mode, out_dt):
    key = (plan, dt_mode, out_dt)
    if key not in _BUILD_CACHE:
        _BUILD_CACHE[key] = _build(plan, dt_mode, out_dt)
    return _BUILD_CACHE[key]


def _route(x, gate_w, top_k):
    """Host routing, matching the reference exactly:
    softmax(x @ gate_w.T) -> top-k (ties -> lower index) -> renormalize."""
    logits = x.astype(np.float64) @ gate_w.astype(np.float64).T
    m = logits.max(axis=-1, keepdims=True)
    p = np.exp(logits - m)
    p /= p.sum(axis=-1, keepdims=True)
    idx = np.argsort(-p, axis=-1, kind="stable")[:, :top_k]          # [T, k]
    vals = np.take_along_axis(p, idx, axis=-1)
    vals = vals / vals.sum(axis=-1, keepdims=True)
    return idx, vals.astype(np.float32)


def _fake_device(in_maps, plan):
    """Numpy stand-in for the device: consumes the exact tiled in_maps
    (validates host-side layouts end-to-end). Dev aid, off by default."""
    class R:
        exec_time_ns = None
        mean_exec_time_ns = None
        results = []
    res = R()
    order = [e for e in range(E) if plan[e][0] > 0]
    for m in in_maps:
        outd = {}
        for e in order:
            nch, w = plan[e]
            C = nch * w
            xs = m[f"xt{e}"].transpose(0, 3, 2, 1).reshape(C, H).astype(
                np.float32)
            w13 = m["w13t"][e]                        # [NFS, P, 2, NH, P]
            w1e = w13[:, :, 0].transpose(0, 3, 2, 1).reshape(FSH, H).astype(
                np.float32)
            w3e = w13[:, :, 1].transpose(0, 3, 2, 1).reshape(FSH, H).astype(
                np.float32)
            w2e = m["w2t"][e].transpose(2, 0, 1, 3).reshape(FSH, H).astype(
                np.float32)
            wgt = m[f"wg{e}"][0]
            h1 = xs @ w1e.T
            h3 = xs @ w3e.T
            inter = (h1 / (1 + np.exp(-h1))) * h3
            y = ((inter @ w2e) * wgt[:, None]).T      # [H, C]
            outd[f"yt{e}"] = np.ascontiguousarray(
                y.reshape(NH // 2, 2, P, C).transpose(0, 2, 1, 3))
        res.results.append(outd)
    return res


def kernel(x, gate_w, w1, w2, w3, top_k):
    from concourse.bass_utils import run_bass_kernel_spmd

    x = np.ascontiguousarray(np.asarray(x, dtype=np.float32))
    gate_w = np.asarray(gate_w, dtype=np.float32)
    w1 = np.asarray(w1, dtype=np.float32)
    w2 = np.asarray(w2, dtype=np.float32)
    w3 = np.asarray(w3, dtype=np.float32)
    k = int(np.asarray(top_k))
    t, h = x.shape
    e_ = gate_w.shape[0]
    f = w1.shape[0] // e_
    assert (h, f, e_) == (H, F, E), (h, f, e_)

    dt_mode = DT_MODE
    import ml_dtypes
    np_mm = {"bf16": ml_dtypes.bfloat16, "fp16": np.float16}.get(
        dt_mode, np.float32)

    idx, vals = _route(x, gate_w, k)                                  # [T, k]

    # token lists per expert
    tok_lists = []
    wgt_lists = []
    for ei in range(E):
        tok_i, slot_i = np.nonzero(idx == ei)
        tok_lists.append(tok_i.astype(np.int64))
        wgt_lists.append(vals[tok_i, slot_i].astype(np.float32))
    plan = tuple(_chunk_plan(len(ti)) for ti in tok_lists)
    order = [ei for ei in range(E) if plan[ei][0] > 0]
    caps = {ei: plan[ei][0] * plan[ei][1] for ei in order}
    offs = {}
    ctot = 0
    for ei in order:
        offs[ei] = ctot
        ctot += caps[ei]

    xmm = x.astype(np_mm)
    shared = {}
    for ei in order:
        nch, w = plan[ei]
        C = caps[ei]
        tok = tok_lists[ei]
        n = len(tok)
        xs = np.zeros((C, H), dtype=np_mm)
        xs[:n] = xmm[tok]
        # xt [nch, P, NH, w] (chunk-major: per-chunk DMAs are contiguous)
        shared[f"xt{ei}"] = np.ascontiguousarray(
            xs.reshape(nch, w, NH, P).transpose(0, 3, 2, 1))
        wgt = np.zeros(C, dtype=np.float32)
        wgt[:n] = wgt_lists[ei]
        shared[f"wg{ei}"] = np.ascontiguousarray(
            np.broadcast_to(wgt, (P, C)).astype(np.float32))

    w1r = w1.reshape(E, F, H)
    w3r = w3.reshape(E, F, H)
    w2r = w2.reshape(E, F, H)
    in_maps = []
    for c in range(NCORES):
        sl = slice(c * FSH, (c + 1) * FSH)
        # [E, FSH, H] -> [E, NFS, P(j), NH(n), P(p)]
        a1 = w1r[:, sl, :].astype(np_mm).reshape(E, NFS, P, NH, P)
        a3 = w3r[:, sl, :].astype(np_mm).reshape(E, NFS, P, NH, P)
        # w13t [E, NFS, P, 2, NH, P]: [e,fi,p,m,n,j]
        w13t = np.ascontiguousarray(np.stack(
            [a1.transpose(0, 1, 4, 3, 2), a3.transpose(0, 1, 4, 3, 2)],
            axis=3))
        # w2t [E, P, NH, NFS, P]: [e,p,ht,fi,j] = w2[e, fi*P+p, ht*P+j]
        b2 = w2r[:, sl, :].astype(np_mm).reshape(E, NFS, P, NH, P)
        w2t = np.ascontiguousarray(b2.transpose(0, 2, 3, 1, 4))
        m = {"w13t": w13t, "w2t": w2t}
        m.update(shared)
        in_maps.append(m)

    if os.environ.get("MOE_FAKE"):
        res = _fake_device(in_maps, plan)
    else:
        nc = _get_nc(plan, dt_mode, OUT_DT)
        trace = bool(int(os.environ.get("MOE_TRACE", "0")))
        res = run_bass_kernel_spmd(nc, in_maps, core_ids=list(range(NCORES)),
                                   trace=trace)
    LAST_STATS.clear()
    LAST_STATS.update({
        "plan": plan,
        "ctot": ctot,
        "dt_mode": dt_mode,
        "out_dt": OUT_DT,
        "exec_time_ns": res.exec_time_ns,
        "mean_exec_time_ns": res.mean_exec_time_ns,
        "counts": [len(ti) for ti in tok_lists],
    })

    # all-reduce the ffn-sharded partials, then scatter-add per token
    out = np.zeros((t, h), dtype=np.float32)
    for ei in order:
        n = len(tok_lists[ei])
        if not n:
            continue
        C = caps[ei]
        ysum = np.zeros((NH // 2, P, 2, C), dtype=np.float32)
        for c in range(NCORES):
            ysum += np.asarray(res.results[c][f"yt{ei}"], dtype=np.float32)
        yfull = ysum.transpose(0, 2, 1, 3).reshape(H, C)
        out[tok_lists[ei]] += yfull[:, :n].T
    return out


# revision 34
# speedup vs baseline: 1.0152x; 1.0152x over previous
"""Block-sparse MoE (SwiGLU, top-k of 8 experts) on 8 Trainium2 NeuronCores.

Sharding: ffn-dimension-parallel (the spec's primary hint). Every core holds
a F/8 = 512-row slice of w1/w3/w2 for ALL 8 experts; x is replicated
(gathered per expert on the host, pre-scaled capacity C_e per expert).
Each core loops over experts e: phase1(e) computes its slice of
inter = silu(w1 x) * (w3 x); phase2(e) computes the partial output
(w2_slice.T @ inter) * wgt. The host sums the 8 partial outputs
(the "all-reduce") and scatter-adds into the full [T, H] result.

Why this beats expert-parallel (core e = expert e): EP pads every core to
max_e count_e (556 slots for seed-0 routing), ffn-parallel gives every core
identical work of sum_e C_e ~ 4104 slots at 1/8 the ffn width - a ~7%
PE-cycle reduction, robust to any routing skew.

Device loop is software-pipelined as [phase1(e); phase2(e-1)] so the
PSUM->SBUF drain of inter(e) hides under a full phase, never bubbling the
PE. Partial outputs are stored as fp16 (quantization ~5e-7 of out scale
per partial; measured end-to-end rel err stays ~5e-4) to halve the output
DMA. Matmuls default to fp16 as in the EP baseline.
All weight/x DRAM->SBUF transfers are host-pre-tiled contiguous DMAs.
"""

import math
import os

import numpy as np

H = 2048            # hidden dim
F = 4096            # ffn dim per expert
E = 8               # experts
NCORES = 8
P = 128             # partitions
NH = H // P         # 16 h-tiles
FSH = F // NCORES   # 512 ffn rows per core
NFS = FSH // P      # 4 f-tiles per expert per core

DT_MODE = os.environ.get("MOE_DT", "fp16")     # fp16 | bf16 | f32r | f32
OUT_DT = os.environ.get("MOE_OUT_DT", "fp16")  # fp16 | f32 partial outputs

# populated by kernel() for test harness introspection
LAST_STATS = {}

_BUILD_CACHE = {}


def _chunk_plan(count):
    """(nch, w): capacity C_e = nch*w >= count, even chunk width w <= 512
    (PSUM bank is 512 fp32; f32r matmuls need an even moving dim)."""
    if count <= 0:
        return (0, 0)
    c_min = max(64, count)
    n = max(1, math.ceil(c_min / 512))
    w = 2 * math.ceil(c_min / (2 * n))
    return (n, w)


def _build(plan, dt_mode, out_dt):
    """Build + compile the per-core Bass program.

    plan: tuple of E (nch, w) chunk shapes, one per expert (nch=0 -> skip).
    """
    import concourse.bacc as bacc
    import concourse.mybir as mybir
    from concourse import tile

    AF = mybir.ActivationFunctionType
    f32 = mybir.dt.float32
    dmm = {
        "bf16": mybir.dt.bfloat16,
        "fp16": mybir.dt.float16,
        "f32": f32,
    }.get(dt_mode, mybir.dt.float32r)
    odt = f32 if out_dt == "f32" else mybir.dt.float16

    order = [e for e in range(E) if plan[e][0] > 0]
    caps = {e: plan[e][0] * plan[e][1] for e in order}
    offs = {}
    ctot = 0
    for e in order:
        offs[e] = ctot
        ctot += caps[e]

    nc = bacc.Bacc("TRN2", target_bir_lowering=False, debug=False)

    # Host-pre-tiled DRAM layouts (every DMA below is fully contiguous):
    #   xt{e}  [nch, P, NH, w]      xt[ci, p, n, c] = x_tok[ci*w+c, n*P+p]
    #   w13t   [E, NFS, P, 2, NH, P] [e,fi,p,m,n,j] = w{1,3}slice[e, fi*P+j, n*P+p]
    #   w2t    [E, P, NH, NFS, P]    [e,p,ht,fi,j]  = w2slice[e, fi*P+p, ht*P+j]
    #   wg{e}  [P, C_e]             broadcast routing weights
    #   yt     [H, Ctot]            partial output, column block off_e per expert
    xt_d = {
        e: nc.dram_tensor(f"xt{e}", [plan[e][0], P, NH, plan[e][1]], dmm,
                          kind="ExternalInput").ap()
        for e in order
    }
    w13_d = nc.dram_tensor("w13t", [E, NFS, P, 2, NH, P], dmm,
                           kind="ExternalInput").ap()
    w2_d = nc.dram_tensor("w2t", [E, P, NH, NFS, P], dmm,
                          kind="ExternalInput").ap()
    wg_d = {
        e: nc.dram_tensor(f"wg{e}", [P, caps[e]], f32,
                          kind="ExternalInput").ap()
        for e in order
    }
    # per-expert outputs [NH/2, P, 2, C_e]: every store DMA is one fully
    # contiguous burst covering TWO h-tiles (row-strided stores into a
    # [H, Ctot] tensor poison the shared DMA engines and starve the
    # weight stream; per-ht stores pay too much SWDGE issue cost)
    y_d = {
        e: nc.dram_tensor(f"yt{e}", [NH // 2, P, 2, caps[e]], odt,
                          kind="ExternalOutput").ap()
        for e in order
    }

    with tile.TileContext(nc) as tc:
        with (
            tc.tile_pool(name="inter", bufs=2) as inter_pool,
            tc.tile_pool(name="psum", bufs=4, space="PSUM") as psum_pool,
            tc.tile_pool(name="misc", bufs=1) as misc_pool,
            tc.tile_pool(name="xtp", bufs=4) as xt_pool,
            tc.tile_pool(name="wcol", bufs=5) as wcol_pool,
            tc.tile_pool(name="p1tmp", bufs=2) as p1tmp,
            tc.tile_pool(name="w2col", bufs=2) as w2_pool,
            tc.tile_pool(name="wgp", bufs=3) as wg_pool,
            # ob ring must cover the SWDGE store latency (~6-12 us) at the
            # ~1.7 us/pair drain cadence, else the DVE drain stalls and
            # the PE blocks on PSUM slot reuse behind it.
            tc.tile_pool(name="obp", bufs=8) as ob_pool,
        ):
            # PE warmup: zero-matmuls with no DMA dependencies run
            # immediately, lifting the HAM clock gate (1.2 -> 2.4 GHz)
            # while the first real loads are still in flight.
            wsrc = misc_pool.tile([P, P], dmm, tag="wsrc")
            nc.vector.memset(wsrc[:], 0.0)
            wps = psum_pool.tile([P, 64], f32, tag="ps3", bufs=4,
                                 name="warm_ps")
            for i in range(80):
                nc.tensor.matmul(wps[:], wsrc[:], wsrc[:, :64],
                                 start=(i == 0), stop=(i == 79))

            # Startup critical path: the SP queue issues its first DMA
            # earliest (~6 us). Interleave the first expert's chunk-a and
            # the first weight column on SP in h-halves so the hi<8
            # matmuls start as soon as the first halves land. The second
            # expert's x rides the ACT HWDGE queue (starts ~9-10 us).
            e0 = order[0]
            xts = {}

            def _xt_tiles(e):
                nch, w = plan[e]
                return [
                    xt_pool.tile([P, NH, w], dmm, tag="xt",
                                 name=f"xt{e}_{ci}")
                    for ci in range(nch)
                ]

            xts[e0] = _xt_tiles(e0)
            wc0 = wcol_pool.tile([P, 2, NH, P], dmm, tag="wc", name="wc0")
            # early-window DMA rate is ~135 GB/s PER QUEUE (instruction
            # fetch streams), and the queues are independent: split the
            # startup-critical path across both HWDGE queues — weights on
            # SP, x on ACT — in graded h-pieces (2,2,4,8 h-tiles) so the
            # first matmul starts after ~0.5 MB and the hi-loop consumes
            # later pieces as they land.
            pieces = [(0, 2), (2, 2), (4, 4), (8, 8)]
            for p0, pw in pieces:
                sl = slice(p0, p0 + pw)
                nc.sync.dma_start(wc0[:, :, sl, :], w13_d[e0][0][:, :, sl, :])
                nc.scalar.dma_start(xts[e0][0][:, sl, :],
                                    xt_d[e0][0][:, sl, :])
            for ci in range(1, plan[e0][0]):
                for sl in (slice(0, 4), slice(4, 10), slice(10, NH)):
                    nc.scalar.dma_start(xts[e0][ci][:, sl, :],
                                        xt_d[e0][ci][:, sl, :])
            if len(order) > 1:
                e1 = order[1]
                xts[e1] = _xt_tiles(e1)
                for ci in range(plan[e1][0]):
                    nc.scalar.dma_start(xts[e1][ci][:], xt_d[e1][ci])

            prev = None  # (e, inter tiles, w2c, wg, chunks)
            wc_anchor = None

            for idx, e in enumerate(order):
                nch, w = plan[e]
                C = caps[e]
                chunks = [(ci * w, w) for ci in range(nch)]

                # prefetch the next expert's x (e0/e1 preloaded at startup)
                if idx + 1 < len(order):
                    en = order[idx + 1]
                    if en not in xts:
                        xts[en] = _xt_tiles(en)
                        for ci in range(plan[en][0]):
                            nc.scalar.dma_start(xts[en][ci][:], xt_d[en][ci])

                # routing weights for this expert (consumed one block later
                # in phase2(e)); SWDGE queue, issued after the fi loop so
                # it never competes with this expert's weight columns.
                wg = wg_pool.tile([P, C], f32, tag="wg", name=f"wg{e}_t")

                # ---- phase 1: inter[f', :] = silu(w1 @ xT) * (w3 @ xT) ----
                its = []
                for fi in range(NFS):
                    if idx == 0 and fi == 0:
                        wc = wc0        # preloaded in the startup block
                    else:
                        wc = wcol_pool.tile([P, 2, NH, P], dmm, tag="wc",
                                            name=f"wc{e}_{fi}")
                        if idx == 0 and fi <= 2:
                            # still inside the slow startup window: land
                            # in quarters so the hi-loop can start early
                            h4 = NH // 4
                            for q in range(4):
                                sl = slice(q * h4, (q + 1) * h4)
                                dma = nc.sync.dma_start(
                                    wc[:, :, sl, :],
                                    w13_d[e][fi][:, :, sl, :])
                                if fi == 2:
                                    wc_anchor = dma
                        else:
                            nc.sync.dma_start(wc[:], w13_d[e][fi])
                    it = inter_pool.tile([P, C], dmm, tag=f"inter{fi}",
                                         name=f"inter{e}_{fi}")
                    its.append(it)
                    if idx == 0 and fi == 0:
                        # chunk-outer: chunk-a's matmuls only wait on the
                        # first x chunk's DMA
                        for ci, (c0, cw) in enumerate(chunks):
                            ps1s = psum_pool.tile([P, cw], f32, tag="ps1",
                                                  bufs=4, name=f"ps1s{ci}")
                            ps3s = psum_pool.tile([P, cw], f32, tag="ps3",
                                                  bufs=4, name=f"ps3s{ci}")
                            for hi in range(NH):
                                nc.tensor.matmul(
                                    ps1s[:], wc[:, 0, hi, :],
                                    xts[e][ci][:, hi, :],
                                    start=(hi == 0), stop=(hi == NH - 1))
                            for hi in range(NH):
                                nc.tensor.matmul(
                                    ps3s[:], wc[:, 1, hi, :],
                                    xts[e][ci][:, hi, :],
                                    start=(hi == 0), stop=(hi == NH - 1))
                            sig = p1tmp.tile([P, cw], f32, tag="sig")
                            nc.scalar.activation(sig[:], ps1s[:], AF.Sigmoid)
                            sil = p1tmp.tile([P, cw], f32, tag="sil")
                            nc.vector.tensor_mul(sil[:], ps1s[:], sig[:])
                            nc.vector.tensor_mul(it[:, c0:c0 + cw], sil[:],
                                                 ps3s[:])
                    else:
                        # interleaved: consecutive matmuls share the
                        # stationary operand across chunks
                        ps1 = [psum_pool.tile([P, cw], f32, tag="ps1",
                                              bufs=4, name=f"ps1_{e}_{fi}_{ci}")
                               for ci, (c0, cw) in enumerate(chunks)]
                        ps3 = [psum_pool.tile([P, cw], f32, tag="ps3",
                                              bufs=4, name=f"ps3_{e}_{fi}_{ci}")
                               for ci, (c0, cw) in enumerate(chunks)]
                        for hi in range(NH):
                            for ci in range(nch):
                                nc.tensor.matmul(
                                    ps1[ci][:], wc[:, 0, hi, :],
                                    xts[e][ci][:, hi, :],
                                    start=(hi == 0), stop=(hi == NH - 1))
                            for ci in range(nch):
                                nc.tensor.matmul(
                                    ps3[ci][:], wc[:, 1, hi, :],
                                    xts[e][ci][:, hi, :],
                                    start=(hi == 0), stop=(hi == NH - 1))
                        for ci, (c0, cw) in enumerate(chunks):
                            sig = p1tmp.tile([P, cw], f32, tag="sig")
                            nc.scalar.activation(sig[:], ps1[ci][:],
                                                 AF.Sigmoid)
                            sil = p1tmp.tile([P, cw], f32, tag="sil")
                            nc.vector.tensor_mul(sil[:], ps1[ci][:], sig[:])
                            nc.vector.tensor_mul(it[:, c0:c0 + cw], sil[:],
                                                 ps3[ci][:])

                # w2 slice for this expert: one contiguous 2.1 MB DMA on
                # the SWDGE (gpsimd) queue. The early-window DMA budget is
                # GLOBAL across queues (~130 GB/s total): w2/wg for the
                # first expert aren't needed until its phase 2 (~65 us),
                # so gate them behind the startup-critical weight columns.
                wg_dma = nc.gpsimd.dma_start(wg[:], wg_d[e][:])
                w2c = w2_pool.tile([P, NH, NFS, P], dmm, tag="w2c",
                                   name=f"w2c{e}")
                w2_dma = nc.gpsimd.dma_start(w2c[:], w2_d[e])
                if idx == 0 and wc_anchor is not None:
                    tile.add_dep_helper(
                        wg_dma.ins, wc_anchor.ins,
                        reason="delay wg past kernel startup")
                    tile.add_dep_helper(
                        w2_dma.ins, wc_anchor.ins,
                        reason="delay w2 load past kernel startup")

                # ---- phase 2 of the previous expert ----
                if prev is not None:
                    _phase2(nc, tc, psum_pool, ob_pool, y_d, prev,
                            last=False, odt=odt)
                prev = (e, its, w2c, wg, chunks, offs[e], C)

            _phase2(nc, tc, psum_pool, ob_pool, y_d, prev, last=True,
                    odt=odt)

    nc.compile()
    return nc


def _phase2(nc, tc, psum_pool, ob_pool, y_d, prev, last, odt):
    """yT[ht, :] = (w2slice.T @ interT) * wgt for one expert."""
    import concourse.mybir as mybir
    f32 = mybir.dt.float32
    e, its, w2c, wg, chunks, off, C = prev
    nch = len(chunks)
    for hp in range(NH // 2):
        ob = ob_pool.tile([P, 2, C], odt, tag="ob", name=f"ob{e}_{hp}")
        for m in range(2):
            ht = 2 * hp + m
            # po shares the (phase-1) ps1 tag: 4 PSUM slots total keep
            # the next group's matmuls from waiting on the drain.
            po = [psum_pool.tile([P, cw], f32, tag="ps1", bufs=4,
                                 name=f"po_{e}_{ht}_{ci}")
                  for ci, (c0, cw) in enumerate(chunks)]
            for fi in range(NFS):
                for ci, (c0, cw) in enumerate(chunks):
                    nc.tensor.matmul(
                        po[ci][:], w2c[:, ht, fi, :],
                        its[fi][:, c0:c0 + cw],
                        start=(fi == 0), stop=(fi == NFS - 1))
            for ci, (c0, cw) in enumerate(chunks):
                nc.vector.tensor_mul(ob[:, m, c0:c0 + cw], po[ci][:],
                                     wg[:, c0:c0 + cw])
            if last and hp >= NH // 2 - 2:
                # final pairs: store per h-tile, alternating queues, so
                # the tail drain overlaps the last matmul groups
                eng = nc.gpsimd if m == 0 else nc.scalar
                eng.dma_start(y_d[e][hp][:, m], ob[:, m])
        if not (last and hp >= NH // 2 - 2):
            # alternate queues: halves each store queue's backlog, so the
            # end-of-kernel queue drains are short
            eng = nc.gpsimd if hp % 2 == 0 else nc.scalar
            eng.dma_start(y_d[e][hp], ob[:])


def _get_nc(plan, dt_mode, out_dt):
    key = (plan, dt_mode, out_dt)
    if key not in _BUILD_CACHE:
        _BUILD_CACHE[key] = _build(plan, dt_mode, out_dt)
    return _BUILD_CACHE[key]


def _route(x, gate_w, top_k):
    """Host routing, matching the reference exactly:
    softmax(x @ gate_w.T) -> top-k (ties -> lower index) -> renormalize."""
    logits = x.astype(np.float64) @ gate_w.astype(np.float64).T
    m = logits.max(axis=-1, keepdims=True)
    p = np.exp(logits - m)
    p /= p.sum(axis=-1, keepdims=True)
    idx = np.argsort(-p, axis=-1, kind="stable")[:, :top_k]          # [T, k]
    vals = np.take_along_axis(p, idx, axis=-1)
    vals = vals / vals.sum(axis=-1, keepdims=True)
    return idx, vals.astype(np.float32)


def _fake_device(in_maps, plan):
    """Numpy stand-in for the device: consumes the exact tiled in_maps
    (validates host-side layouts end-to-end). Dev aid, off by default."""
    class R:
        exec_time_ns = None
        mean_exec_time_ns = None
        results = []
    res = R()
    order = [e for e in range(E) if plan[e][0] > 0]
    for m in in_maps:
        outd = {}
        for e in order:
            nch, w = plan[e]
            C = nch * w
            xs = m[f"xt{e}"].transpose(0, 3, 2, 1).reshape(C, H).astype(
                np.float32)
            w13 = m["w13t"][e]                        # [NFS, P, 2, NH, P]
            w1e = w13[:, :, 0].transpose(0, 3, 2, 1).reshape(FSH, H).astype(
                np.float32)
            w3e = w13[:, :, 1].transpose(0, 3, 2, 1).reshape(FSH, H).astype(
                np.float32)
            w2e = m["w2t"][e].transpose(2, 0, 1, 3).reshape(FSH, H).astype(
                np.float32)
            wgt = m[f"wg{e}"][0]
            h1 = xs @ w1e.T
            h3 = xs @ w3e.T
            inter = (h1 / (1 + np.exp(-h1))) * h3
            y = ((inter @ w2e) * wgt[:, None]).T      # [H, C]
            outd[f"yt{e}"] = np.ascontiguousarray(
                y.reshape(NH // 2, 2, P, C).transpose(0, 2, 1, 3))
        res.results.append(outd)
    return res


def kernel(x, gate_w, w1, w2, w3, top_k):
    from concourse.bass_utils import run_bass_kernel_spmd

    x = np.ascontiguousarray(np.asarray(x, dtype=np.float32))
    gate_w = np.asarray(gate_w, dtype=np.float32)
    w1 = np.asarray(w1, dtype=np.float32)
    w2 = np.asarray(w2, dtype=np.float32)
    w3 = np.asarray(w3, dtype=np.float32)
    k = int(np.asarray(top_k))
    t, h = x.shape
    e_ = gate_w.shape[0]
    f = w1.shape[0] // e_
    assert (h, f, e_) == (H, F, E), (h, f, e_)

    dt_mode = DT_MODE
    import ml_dtypes
    np_mm = {"bf16": ml_dtypes.bfloat16, "fp16": np.float16}.get(
        dt_mode, np.float32)

    idx, vals = _route(x, gate_w, k)                                  # [T, k]

    # token lists per expert
    tok_lists = []
    wgt_lists = []
    for ei in range(E):
        tok_i, slot_i = np.nonzero(idx == ei)
        tok_lists.append(tok_i.astype(np.int64))
        wgt_lists.append(vals[tok_i, slot_i].astype(np.float32))
    plan = tuple(_chunk_plan(len(ti)) for ti in tok_lists)
    order = [ei for ei in range(E) if plan[ei][0] > 0]
    caps = {ei: plan[ei][0] * plan[ei][1] for ei in order}
    offs = {}
    ctot = 0
    for ei in order:
        offs[ei] = ctot
        ctot += caps[ei]

    xmm = x.astype(np_mm)
    shared = {}
    for ei in order:
        nch, w = plan[ei]
        C = caps[ei]
        tok = tok_lists[ei]
        n = len(tok)
        xs = np.zeros((C, H), dtype=np_mm)
        xs[:n] = xmm[tok]
        # xt [nch, P, NH, w] (chunk-major: per-chunk DMAs are contiguous)
        shared[f"xt{ei}"] = np.ascontiguousarray(
            xs.reshape(nch, w, NH, P).transpose(0, 3, 2, 1))
        wgt = np.zeros(C, dtype=np.float32)
        wgt[:n] = wgt_lists[ei]
        shared[f"wg{ei}"] = np.ascontiguousarray(
            np.broadcast_to(wgt, (P, C)).astype(np.float32))

    w1r = w1.reshape(E, F, H)
    w3r = w3.reshape(E, F, H)
    w2r = w2.reshape(E, F, H)
    in_maps = []
    for c in range(NCORES):
        sl = slice(c * FSH, (c + 1) * FSH)
        # [E, FSH, H] -> [E, NFS, P(j), NH(n), P(p)]
        a1 = w1r[:, sl, :].astype(np_mm).reshape(E, NFS, P, NH, P)
        a3 = w3r[:, sl, :].astype(np_mm).reshape(E, NFS, P, NH, P)
        # w13t [E, NFS, P, 2, NH, P]: [e,fi,p,m,n,j]
        w13t = np.ascontiguousarray(np.stack(
            [a1.transpose(0, 1, 4, 3, 2), a3.transpose(0, 1, 4, 3, 2)],
            axis=3))
        # w2t [E, P, NH, NFS, P]: [e,p,ht,fi,j] = w2[e, fi*P+p, ht*P+j]
        b2 = w2r[:, sl, :].astype(np_mm).reshape(E, NFS, P, NH, P)
        w2t = np.ascontiguousarray(b2.transpose(0, 2, 3, 1, 4))
        m = {"w13t": w13t, "w2t": w2t}
        m.update(shared)
        in_maps.append(m)

    if os.environ.get("MOE_FAKE"):
        res = _fake_device(in_maps, plan)
    else:
        nc = _get_nc(plan, dt_mode, OUT_DT)
        trace = bool(int(os.environ.get("MOE_TRACE", "0")))
        res = run_bass_kernel_spmd(nc, in_maps, core_ids=list(range(NCORES)),
                                   trace=trace)
    LAST_STATS.clear()
    LAST_STATS.update({
        "plan": plan,
        "ctot": ctot,
        "dt_mode": dt_mode,
        "out_dt": OUT_DT,
        "exec_time_ns": res.exec_time_ns,
        "mean_exec_time_ns": res.mean_exec_time_ns,
        "counts": [len(ti) for ti in tok_lists],
    })

    # all-reduce the ffn-sharded partials, then scatter-add per token
    out = np.zeros((t, h), dtype=np.float32)
    for ei in order:
        n = len(tok_lists[ei])
        if not n:
            continue
        C = caps[ei]
        ysum = np.zeros((NH // 2, P, 2, C), dtype=np.float32)
        for c in range(NCORES):
            ysum += np.asarray(res.results[c][f"yt{ei}"], dtype=np.float32)
        yfull = ysum.transpose(0, 2, 1, 3).reshape(H, C)
        out[tok_lists[ei]] += yfull[:, :n].T
    return out
